# revision 1
# baseline (speedup 1.0000x reference)
"""GQA attention (16 q-heads / 4 kv-heads, head_dim 64, T=2048, D=1024) on 8
Trainium2 NeuronCores.

Sharding: 8 shards = batch(2) x kv-group(4). Each core handles one batch
element and one whole GQA group (4 query heads + their shared kv head), and
computes a partial output projection; the host sums the 4 group-partials per
batch element.

Self-contained: hardcodes all shapes; inputs are the full unsharded tensors.
"""
import sys

if "/opt/trn_rl_repo" not in sys.path:
    sys.path.insert(0, "/opt/trn_rl_repo")

import numpy as np
import ml_dtypes

T = 2048
D = 1024
HD = 64
NH = 4          # q heads per core
TT = 16         # t-tiles of 128
NKT = 16        # k-tiles of 128
W = 1024        # q-chunk width in attention
EPS = 1e-6

_CACHE = {}


def _build_nc(variant="full"):
    import concourse.bass as bass
    import concourse.tile as tile
    from concourse import bacc, mybir
    from concourse.masks import make_identity

    F32 = mybir.dt.float32
    BF16 = mybir.dt.bfloat16
    AF = mybir.ActivationFunctionType
    AX = mybir.AxisListType

    nc = bacc.Bacc("TRN2", target_bir_lowering=False, debug=False,
                   num_devices=8)

    xT_d = nc.dram_tensor("xT", [D, T], BF16, kind="ExternalInput")
    wqkvT_d = nc.dram_tensor("wqkvT", [D, 384], BF16, kind="ExternalInput")
    woT_d = nc.dram_tensor("woT", [256, D], BF16, kind="ExternalInput")
    cc_d = nc.dram_tensor("cc", [T, 320], F32, kind="ExternalInput")
    ss_d = nc.dram_tensor("ss", [T, 320], F32, kind="ExternalInput")
    out_d = nc.dram_tensor("out", [T, D], F32, kind="ExternalOutput")

    with tile.TileContext(nc) as tc:
        with tc.tile_pool(name="singles", bufs=1) as singles:
            # --- persistent SBUF tensors ---
            ident = singles.tile([128, 128], BF16)
            make_identity(nc, ident)
            ones = singles.tile([128, 64], BF16)
            nc.vector.memset(ones, 1.0)
            # causal mask M[p, c] = 1.0 if p <= c - 512 else 0.0
            M = singles.tile([128, 1536], BF16)
            nc.gpsimd.memset(M, 1.0)
            nc.gpsimd.affine_select(
                out=M, in_=M, compare_op=mybir.AluOpType.is_ge, fill=0.0,
                base=-512, channel_multiplier=-1, pattern=[[1, 1536]])

            wqkv = singles.tile([128, 8, 384], BF16)
            nc.sync.dma_start(
                out=wqkv, in_=wqkvT_d[:, :].rearrange("(c p) n -> p c n", p=128))
            wo = singles.tile([128, 2, 1024], BF16)
            nc.sync.dma_start(
                out=wo, in_=woT_d[:, :].rearrange("(c p) n -> p c n", p=128))

            xT = []
            for c in range(8):
                xt = singles.tile([128, T], BF16, tag=f"xT{c}")
                nc.sync.dma_start(
                    out=xt, in_=xT_d[128 * c:128 * (c + 1), :])
                xT.append(xt)

            qT = singles.tile([128, 2, T], BF16)       # 4 heads: (h%2)*64+dh, h//2
            kT = singles.tile([128, T], BF16)          # kv head dims duplicated 2x
            v1 = singles.tile([128, TT, 65], BF16)     # V tiles + ones col
            nc.vector.memset(v1[:, :, 64:65], 1.0)
            attnT = singles.tile([128, 2, T], BF16)    # normalized attn out^T
            eps_t = singles.tile([128, 1], F32)
            nc.vector.memset(eps_t, EPS)

            # ---------------- Phase B: proj + rmsnorm + rope + transpose -----
            with tc.tile_pool(name="ps_proj", bufs=2, space="PSUM") as ps_proj, \
                 tc.tile_pool(name="ps_tr", bufs=3, space="PSUM") as ps_tr, \
                 tc.tile_pool(name="bwork", bufs=3) as bwork, \
                 tc.tile_pool(name="bsmall", bufs=3) as bsmall:
                for i in range(TT):
                    ts = slice(128 * i, 128 * (i + 1))
                    pq = ps_proj.tile([128, 384], F32, tag="pq")
                    for c in range(8):
                        nc.tensor.matmul(
                            pq, xT[c][:, ts], wqkv[:, c, :],
                            start=(c == 0), stop=(c == 7))
                    qk = bwork.tile([128, 320], F32, tag="qk")
                    nc.scalar.copy(qk, pq[:, 0:320])
                    nc.scalar.copy(v1[:, i, 0:64], pq[:, 320:384])
                    sq = bwork.tile([128, 320], F32, tag="sq")
                    nc.vector.tensor_mul(sq, qk, qk)
                    ssum = bsmall.tile([128, 5], F32, tag="ssum")
                    nc.vector.reduce_sum(
                        out=ssum, in_=sq.rearrange("p (h d) -> p h d", h=5),
                        axis=AX.X)
                    stdv = bsmall.tile([128, 5], F32, tag="stdv")
                    nc.scalar.activation(
                        out=stdv, in_=ssum, func=AF.Sqrt, bias=eps_t[:, :],
                        scale=1.0 / HD)
                    rstd = bsmall.tile([128, 5], F32, tag="rstd")
                    nc.vector.reciprocal(rstd, stdv)
                    for hh in range(5):
                        nc.vector.tensor_scalar_mul(
                            qk[:, 64 * hh:64 * (hh + 1)],
                            qk[:, 64 * hh:64 * (hh + 1)],
                            rstd[:, hh:hh + 1])
                    cs = bwork.tile([128, 320], F32, tag="cs")
                    nc.sync.dma_start(out=cs, in_=cc_d[ts, :])
                    sn = bwork.tile([128, 320], F32, tag="sn")
                    nc.sync.dma_start(out=sn, in_=ss_d[ts, :])
                    xc = bwork.tile([128, 320], F32, tag="xc")
                    nc.vector.tensor_mul(xc, qk, cs)
                    # swap halves within each 64-block: [x2 | x1]
                    import concourse.bass as bass_mod
                    qk_swap = bass_mod.AP(
                        tensor=qk.tensor, offset=qk.offset + 32,
                        ap=[qk.ap[0], [64, 5], [-32, 2], [1, 32]])
                    xs = bwork.tile([128, 5, 64], F32, tag="xs")
                    nc.vector.tensor_mul(xs, qk_swap, sn.rearrange(
                        "p (h d) -> p h d", h=5))
                    rope = bwork.tile([128, 320], BF16, tag="rope")
                    nc.vector.tensor_add(
                        rope, xc, xs.rearrange("p h d -> p (h d)"))
                    ktr = bwork.tile([128, 128], BF16, tag="ktr")
                    nc.gpsimd.tensor_copy(ktr[:, 0:64], rope[:, 256:320])
                    nc.gpsimd.tensor_copy(ktr[:, 64:128], rope[:, 256:320])
                    for pair in range(2):
                        tp = ps_tr.tile([128, 128], BF16, tag="tp")
                        nc.tensor.transpose(
                            tp, rope[:, 128 * pair:128 * (pair + 1)],
                            ident)
                        nc.scalar.copy(qT[:, pair, ts], tp)
                    tpk = ps_tr.tile([128, 128], BF16, tag="tp")
                    nc.tensor.transpose(tpk, ktr, ident)
                    nc.scalar.copy(kT[:, ts], tpk)

            if variant == "proj":
                nc.gpsimd.dma_start(out=out_d[0:128, 0:D],
                                    in_=qT[:, 0, 0:D])
            # ---------------- Phase C: attention ----------------------------
            with tc.tile_pool(name="ps_sc", bufs=2, space="PSUM") as ps_sc, \
                 tc.tile_pool(name="ps_pv", bufs=2, space="PSUM") as ps_pv, \
                 tc.tile_pool(name="cwork", bufs=3) as cwork, \
                 tc.tile_pool(name="cnorm", bufs=2) as cnorm:
                for qc in range(2):
                    for h in range(NH):
                        hp = 64 * (h % 2)
                        hc = h // 2
                        n_kt = 8 * qc + 8
                        pv = ps_pv.tile([65, W], F32, tag="pv")
                        for kt in range(n_kt):
                            blo = 0 if kt < 8 * qc + 4 else 512
                            delta = 128 * kt - W * qc
                            lo = max(blo, delta)
                            sc = ps_sc.tile([128, W], F32, tag="sc")
                            for b in range(2):
                                if 128 * kt < W * qc + 512 * (b + 1):
                                    nc.tensor.matmul(
                                        sc[:, 512 * b:512 * (b + 1)],
                                        kT[hp:hp + 64,
                                           128 * kt:128 * (kt + 1)],
                                        qT[hp:hp + 64, hc,
                                           W * qc + 512 * b:
                                           W * qc + 512 * (b + 1)],
                                        start=True, stop=True)
                            es = cwork.tile([128, W], BF16, tag="expS")
                            if lo > blo:
                                nc.vector.memset(es[:, blo:lo], 0.0)
                            nc.scalar.activation(
                                out=es[:, lo:W], in_=sc[:, lo:W], func=AF.Exp)
                            if kt >= 8 * qc:  # diagonal: causal mask
                                nc.vector.tensor_mul(
                                    es[:, blo:W], es[:, blo:W],
                                    M[:, 512 + blo - delta:512 + W - delta])
                            for b in range(2):
                                if 128 * kt < W * qc + 512 * (b + 1):
                                    last = kt == min(n_kt, 8 * qc + 4 * (b + 1)) - 1
                                    nc.tensor.matmul(
                                        pv[:, 512 * b:512 * (b + 1)],
                                        v1[:, kt, :],
                                        es[:, 512 * b:512 * (b + 1)],
                                        start=(kt == 0), stop=last)
                        rc = cnorm.tile([128, W], BF16, tag="rc")
                        with nc.allow_low_precision(reason="f32r out, fp32 bytes"):
                            nc.vector.reciprocal(rc[64:65, :], pv[64:65, :])
                        bc = ps_sc.tile([64, W], F32, tag="sc")
                        for b in range(2):
                            nc.tensor.matmul(
                                bc[:, 512 * b:512 * (b + 1)],
                                ones[64:65, 0:64],
                                rc[64:65, 512 * b:512 * (b + 1)],
                                start=True, stop=True)
                        pvc = cnorm.tile([64, W], F32, tag="pvc")
                        nc.vector.tensor_copy(pvc, pv[0:64, :])
                        nc.vector.tensor_mul(
                            attnT[hp:hp + 64, hc, W * qc:W * (qc + 1)],
                            pvc, bc)

            if variant == "attn":
                nc.gpsimd.dma_start(out=out_d[0:128, 0:D],
                                    in_=attnT[:, 0, 0:D])
            # ---------------- Phase D: output projection ---------------------
            with tc.tile_pool(name="ps_o", bufs=2, space="PSUM") as ps_o, \
                 tc.tile_pool(name="dwork", bufs=3) as dwork:
                for i in range(TT):
                    ts = slice(128 * i, 128 * (i + 1))
                    po = ps_o.tile([128, 1024], F32, tag="po")
                    for nh in range(2):
                        for c in range(2):
                            nc.tensor.matmul(
                                po[:, 512 * nh:512 * (nh + 1)],
                                attnT[:, c, ts],
                                wo[:, c, 512 * nh:512 * (nh + 1)],
                                start=(c == 0), stop=(c == 1))
                    ob = dwork.tile([128, 1024], F32, tag="ob")
                    nc.scalar.copy(ob[:, 0:512], po[:, 0:512])
                    nc.vector.tensor_copy(ob[:, 512:1024], po[:, 512:1024])
                    nc.sync.dma_start(out=out_d[ts, :], in_=ob)
    nc.compile()
    return nc


def _host_tables(cos, sin, qn_w, kn_w):
    scale = HD ** -0.5
    cch = np.concatenate([cos, cos], 1).astype(np.float32)         # (T, 64)
    ssh = np.concatenate([-sin, sin], 1).astype(np.float32)
    qn4 = np.tile(qn_w, 4).astype(np.float32)
    swq4 = np.tile(np.concatenate([qn_w[32:], qn_w[:32]]), 4).astype(np.float32)
    swk = np.concatenate([kn_w[32:], kn_w[:32]]).astype(np.float32)
    cc = np.concatenate(
        [np.tile(cch, (1, NH)) * qn4[None] * scale, cch * kn_w[None]], 1)
    ss = np.concatenate(
        [np.tile(ssh, (1, NH)) * swq4[None] * scale, ssh * swk[None]], 1)
    return np.ascontiguousarray(cc, np.float32), np.ascontiguousarray(ss, np.float32)


def make_in_maps(x, cos, sin, wq, wk, wv, wo, qn_w, kn_w):
    cc, ss = _host_tables(cos, sin, qn_w, kn_w)
    in_maps = []
    for core in range(8):
        b, g = divmod(core, 4)
        wqkvT = np.ascontiguousarray(np.concatenate(
            [wq[256 * g:256 * (g + 1)],
             wk[64 * g:64 * (g + 1)],
             wv[64 * g:64 * (g + 1)]], 0).T.astype(ml_dtypes.bfloat16))
        woT = np.ascontiguousarray(
            wo[:, 256 * g:256 * (g + 1)].T.astype(ml_dtypes.bfloat16))
        xT = np.ascontiguousarray(np.asarray(x)[b].T.astype(ml_dtypes.bfloat16))
        in_maps.append({"xT": xT, "wqkvT": wqkvT, "woT": woT,
                        "cc": cc, "ss": ss})
    return in_maps


def kernel(x, cos, sin, wq, wk, wv, wo, qn_w, kn_w):
    from concourse.bass_utils import run_bass_kernel_spmd

    if "nc" not in _CACHE:
        _CACHE["nc"] = _build_nc()
    nc = _CACHE["nc"]
    in_maps = make_in_maps(np.asarray(x), np.asarray(cos), np.asarray(sin),
                           np.asarray(wq), np.asarray(wk), np.asarray(wv),
                           np.asarray(wo), np.asarray(qn_w), np.asarray(kn_w))
    res = run_bass_kernel_spmd(nc, in_maps, core_ids=list(range(8)))
    out = np.zeros((2, T, D), np.float32)
    for core in range(8):
        b = core // 4
        out[b] += res.results[core]["out"]
    return out



# revision 12
# speedup vs baseline: 1.6179x; 1.6179x over previous
"""GQA attention (16 q-heads / 4 kv-heads, head_dim 64, T=2048, D=1024) on 8
Trainium2 NeuronCores.

Sharding: 8 shards = batch(2) x kv-group(4). Each core handles one batch
element and one whole GQA group (4 query heads + their shared kv head), and
computes a partial output projection; the host sums the 4 group-partials per
batch element.

Self-contained: hardcodes all shapes; inputs are the full unsharded tensors.
"""
import sys

if "/opt/trn_rl_repo" not in sys.path:
    sys.path.insert(0, "/opt/trn_rl_repo")

import numpy as np
import ml_dtypes

T = 2048
D = 1024
HD = 64
NH = 4          # q heads per core
TT = 16         # t-tiles of 128
EPS = 1e-6

_CACHE = {}


def _build_nc(variant="full"):
    import concourse.bass as bass
    import concourse.tile as tile
    from concourse import bacc, mybir
    from concourse.masks import make_identity

    F32 = mybir.dt.float32
    BF16 = mybir.dt.bfloat16
    AF = mybir.ActivationFunctionType
    AX = mybir.AxisListType

    nc = bacc.Bacc("TRN2", target_bir_lowering=False, debug=False,
                   num_devices=8)

    xT_d = nc.dram_tensor("xT", [D, T], BF16, kind="ExternalInput")
    wqkvT_d = nc.dram_tensor("wqkvT", [D, 384], BF16, kind="ExternalInput")
    woT_d = nc.dram_tensor("woT", [256, D], BF16, kind="ExternalInput")
    cc_d = nc.dram_tensor("cc", [T, 320], BF16, kind="ExternalInput")
    ss_d = nc.dram_tensor("ss", [T, 320], BF16, kind="ExternalInput")
    out_d = nc.dram_tensor("out", [T, D], BF16, kind="ExternalOutput")

    with tile.TileContext(nc) as tc:
        with tc.tile_pool(name="singles", bufs=1) as singles:
            # --- persistent SBUF tensors ---
            ident = singles.tile([128, 128], BF16)
            make_identity(nc, ident)
            # wedge mask M[p, c] = 1.0 if p <= c else 0.0
            M = singles.tile([128, 128], BF16)
            nc.gpsimd.memset(M, 1.0)
            nc.gpsimd.affine_select(
                out=M, in_=M, compare_op=mybir.AluOpType.is_ge, fill=0.0,
                base=0, channel_multiplier=-1, pattern=[[1, 128]])

            wqkv = singles.tile([128, 8, 384], BF16)
            nc.sync.dma_start(
                out=wqkv, in_=wqkvT_d[:, :].rearrange("(c p) n -> p c n", p=128))
            wo = singles.tile([128, 2, 1024], BF16)
            nc.sync.dma_start(
                out=wo, in_=woT_d[:, :].rearrange("(c p) n -> p c n", p=128))

            xT = []
            for c in range(8):
                xt = singles.tile([128, T], BF16, tag=f"xT{c}")
                xT.append(xt)
            for half in range(2):
                hs = slice(1024 * half, 1024 * (half + 1))
                for c in range(8):
                    nc.sync.dma_start(
                        out=xT[c][:, hs],
                        in_=xT_d[128 * c:128 * (c + 1), hs])

            # qkT[:, 0, :] = heads {0,2} dims, qkT[:, 1, :] = heads {1,3},
            # qkT[:, 2, :] = kv head dims duplicated 2x
            qkT = singles.tile([128, 3, T], BF16)
            v1 = singles.tile([128, TT, 65], BF16)     # V tiles + ones col
            nc.vector.memset(v1[:, :, 64:65], 1.0)
            attnT = singles.tile([128, 2, T], BF16)    # normalized attn out^T
            eps_t = singles.tile([128, 1], F32)
            nc.vector.memset(eps_t, EPS)

            # ---------------- Phase B: proj + rmsnorm + rope + transpose -----
            with tc.tile_pool(name="ps_proj", bufs=3, space="PSUM") as ps_proj, \
                 tc.tile_pool(name="ps_tr", bufs=3, space="PSUM") as ps_tr, \
                 tc.tile_pool(name="bwork", bufs=4) as bwork, \
                 tc.tile_pool(name="bsmall", bufs=4) as bsmall:
                for i in range(TT):
                    ts = slice(128 * i, 128 * (i + 1))
                    pq = ps_proj.tile([128, 384], F32, tag="pq")
                    for c in range(8):
                        nc.tensor.matmul(
                            pq, xT[c][:, ts], wqkv[:, c, :],
                            start=(c == 0), stop=(c == 7))
                    # rms stats: square on ACT (DVE can't dual-read PSUM)
                    sq = bwork.tile([128, 320], F32, tag="sq")
                    nc.scalar.activation(out=sq, in_=pq[:, 0:320],
                                         func=AF.Square)
                    ssum = bsmall.tile([128, 5], F32, tag="ssum")
                    nc.vector.reduce_sum(
                        out=ssum, in_=sq.rearrange("p (h d) -> p h d", h=5),
                        axis=AX.X)
                    stdv = bsmall.tile([128, 5], F32, tag="stdv")
                    nc.scalar.activation(
                        out=stdv, in_=ssum, func=AF.Sqrt, bias=eps_t[:, :],
                        scale=1.0 / HD)
                    rstd = bsmall.tile([128, 5], F32, tag="rstd")
                    nc.vector.reciprocal_approx_fast(out=rstd, in_=stdv)
                    # scaled q/k in bf16 via 0-stride broadcast of rstd
                    rstd_b = bass.AP(
                        tensor=rstd.tensor, offset=rstd.offset,
                        ap=[rstd.ap[0], [1, 5], [0, 64]])
                    qs = bwork.tile([128, 320], BF16, tag="qs")
                    nc.vector.tensor_mul(qs, pq[:, 0:320], rstd_b)
                    # rope (all-bf16)
                    cs = bwork.tile([128, 320], BF16, tag="cs")
                    nc.sync.dma_start(out=cs, in_=cc_d[ts, :])
                    sn = bwork.tile([128, 320], BF16, tag="sn")
                    nc.sync.dma_start(out=sn, in_=ss_d[ts, :])
                    xc = bwork.tile([128, 320], BF16, tag="xc")
                    nc.vector.tensor_mul(xc, qs, cs)
                    qs_swap = bass.AP(
                        tensor=qs.tensor, offset=qs.offset + 32,
                        ap=[qs.ap[0], [64, 5], [-32, 2], [1, 32]])
                    xs = bwork.tile([128, 5, 64], BF16, tag="xs")
                    nc.vector.tensor_mul(xs, qs_swap, sn.rearrange(
                        "p (h d) -> p h d", h=5))
                    rope = bwork.tile([128, 320], BF16, tag="rope")
                    nc.vector.tensor_add(
                        rope, xc, xs.rearrange("p h d -> p (h d)"))
                    ktr = bwork.tile([128, 128], BF16, tag="ktr")
                    nc.gpsimd.tensor_copy(ktr[:, 0:64], rope[:, 256:320])
                    nc.gpsimd.tensor_copy(ktr[:, 64:128], rope[:, 256:320])
                    # 3 transposes into one psum tile, one fused copy out
                    tp = ps_tr.tile([128, 384], BF16, tag="tp")
                    nc.tensor.transpose(tp[:, 0:128], rope[:, 0:128], ident)
                    nc.tensor.transpose(tp[:, 128:256], rope[:, 128:256], ident)
                    nc.tensor.transpose(tp[:, 256:384], ktr, ident)
                    nc.scalar.copy(qkT[:, :, ts], tp.rearrange(
                        "p (c n) -> p c n", c=3))
                    nc.scalar.copy(v1[:, i, 0:64], pq[:, 320:384])

            if variant == "proj":
                nc.gpsimd.dma_start(out=out_d[0:128, 0:D],
                                    in_=qkT[:, 0, 0:D])
            # ---------------- Phase C: attention ----------------------------
            # per (h, kt): score windows of <=1024 cols covering
            # [512*(kt>>2), 2048); exact-causal col starts at 128*kt.
            # pv[j] accumulates chunk j = cols [512j, 512j+512) over kt<=4j+3.
            with tc.tile_pool(name="ps_sc", bufs=2, space="PSUM") as ps_sc, \
                 tc.tile_pool(name="ps_pv", bufs=1, space="PSUM") as ps_pv, \
                 tc.tile_pool(name="es_pool", bufs=6) as es_pool, \
                 tc.tile_pool(name="cnorm", bufs=2) as cnorm, \
                 tc.tile_pool(name="dscr", bufs=2, space="DRAM") as dscr:
                for h in range(NH):
                    hp = 64 * (h % 2)
                    hc = h // 2
                    pv_t = {}
                    es_ref = {}   # (kt, j) -> (es_tile, local col offset)
                    for kti in range(18):
                        # ---- scores + exp for kt = kti ----
                        if kti < 16:
                            kt = kti
                            j0 = kt >> 2
                            base = 512 * j0
                            q0 = 128 * kt          # first valid q col
                            wstarts = [base] if base + 1024 >= 2048 \
                                else [base, base + 1024]
                            for ws in wstarts:
                                we = min(ws + 1024, 2048)
                                sc = ps_sc.tile([128, 1024], F32, tag="sc")
                                es = es_pool.tile([128, 1024], BF16, tag="es")
                                lo = max(ws, q0)
                                # matmuls split at 512 boundaries (psum bank)
                                mstart = lo
                                while mstart < we:
                                    mend = min((mstart // 512 + 1) * 512, we)
                                    nc.tensor.matmul(
                                        sc[:, mstart - ws:mend - ws],
                                        qkT[hp:hp + 64, 2,
                                            128 * kt:128 * (kt + 1)],
                                        qkT[hp:hp + 64, hc, mstart:mend],
                                        start=True, stop=True)
                                    mstart = mend
                                nc.scalar.activation(
                                    out=es[:, lo - ws:we - ws],
                                    in_=sc[:, lo - ws:we - ws], func=AF.Exp)
                                if lo == q0:  # wedge tile lives here
                                    nc.vector.tensor_mul(
                                        es[:, lo - ws:lo - ws + 128],
                                        es[:, lo - ws:lo - ws + 128], M)
                                for j in range(max(j0, ws // 512),
                                               (we + 511) // 512):
                                    es_ref[(kt, j)] = (es, 512 * j - ws)
                        # ---- pv for kt = kti - 2 ----
                        if kti >= 2:
                            kp = kti - 2
                            j0p = kp >> 2
                            if kp == 0:
                                for j in range(4):
                                    pv_t[j] = ps_pv.tile(
                                        [65, 512], F32, tag=f"pv{j}",
                                        name=f"pv{j}")
                            for j in range(j0p, 4):
                                es, off = es_ref[(kp, j)]
                                w0c = 128 * (kp - 4 * j) if j == j0p else 0
                                w0c = max(w0c, 0)
                                last = (kp == 4 * j + 3)
                                nc.tensor.matmul(
                                    pv_t[j][:, w0c:512],
                                    v1[:, kp, :],
                                    es[:, off + w0c:off + 512],
                                    start=(kp == 0), stop=last,
                                    skip_group_check=not (kp == 0 or last))
                            # normalization for finished chunk
                            if kp % 4 == 3:
                                j = kp >> 2
                                dcp = cnorm.tile([1, 512], F32, tag="dcp")
                                nc.vector.tensor_copy(dcp, pv_t[j][64:65, :])
                                rc = cnorm.tile([1, 512], F32, tag="rc")
                                nc.vector.reciprocal_approx_fast(
                                    out=rc, in_=dcp)
                                scr = dscr.tile([1, 512], F32, tag="scr",
                                                name="scr")
                                nc.sync.dma_start(out=scr, in_=rc)
                                rcb = cnorm.tile([64, 512], F32, tag="rcb")
                                src = bass.AP(
                                    tensor=scr.tensor, offset=scr.offset,
                                    ap=[[0, 64], [1, 512]])
                                nc.sync.dma_start(out=rcb, in_=src)
                                nc.vector.tensor_mul(
                                    attnT[hp:hp + 64, hc,
                                          512 * j:512 * (j + 1)],
                                    pv_t[j][0:64, :], rcb)

            if variant == "attn":
                nc.gpsimd.dma_start(out=out_d[0:128, 0:D],
                                    in_=attnT[:, 0, 0:D])
            # ---------------- Phase D: output projection ---------------------
            with tc.tile_pool(name="ps_o", bufs=3, space="PSUM") as ps_o, \
                 tc.tile_pool(name="dwork", bufs=3) as dwork:
                for i in range(TT):
                    ts = slice(128 * i, 128 * (i + 1))
                    ob = dwork.tile([128, 1024], BF16, tag="ob")
                    for nh in range(2):
                        po = ps_o.tile([128, 512], F32, tag="po")
                        for c in range(2):
                            nc.tensor.matmul(
                                po,
                                attnT[:, c, ts],
                                wo[:, c, 512 * nh:512 * (nh + 1)],
                                start=(c == 0), stop=(c == 1))
                        if nh == 0:
                            nc.scalar.copy(ob[:, 0:512], po)
                        else:
                            nc.vector.tensor_copy(ob[:, 512:1024], po)
                    nc.sync.dma_start(out=out_d[ts, :], in_=ob)
            if variant == "dump":
                nc.gpsimd.dma_start(out=out_d[0:128, 0:D],
                                    in_=qkT[:, 0, 0:D])
                nc.gpsimd.dma_start(out=out_d[128:256, 0:D],
                                    in_=qkT[:, 2, 0:D])
                nc.gpsimd.dma_start(out=out_d[256:384, 0:D],
                                    in_=attnT[:, 0, 0:D])
    nc.compile()
    return nc


def _host_tables(cos, sin, qn_w, kn_w):
    scale = HD ** -0.5
    cch = np.concatenate([cos, cos], 1).astype(np.float32)         # (T, 64)
    ssh = np.concatenate([-sin, sin], 1).astype(np.float32)
    qn4 = np.tile(qn_w, 4).astype(np.float32)
    swq4 = np.tile(np.concatenate([qn_w[32:], qn_w[:32]]), 4).astype(np.float32)
    swk = np.concatenate([kn_w[32:], kn_w[:32]]).astype(np.float32)
    cc = np.concatenate(
        [np.tile(cch, (1, NH)) * qn4[None] * scale, cch * kn_w[None]], 1)
    ss = np.concatenate(
        [np.tile(ssh, (1, NH)) * swq4[None] * scale, ssh * swk[None]], 1)
    return (np.ascontiguousarray(cc).astype(ml_dtypes.bfloat16),
            np.ascontiguousarray(ss).astype(ml_dtypes.bfloat16))


def make_in_maps(x, cos, sin, wq, wk, wv, wo, qn_w, kn_w):
    cc, ss = _host_tables(cos, sin, qn_w, kn_w)
    in_maps = []
    for core in range(8):
        b, g = divmod(core, 4)
        wqkvT = np.ascontiguousarray(np.concatenate(
            [wq[256 * g:256 * (g + 1)],
             wk[64 * g:64 * (g + 1)],
             wv[64 * g:64 * (g + 1)]], 0).T.astype(ml_dtypes.bfloat16))
        woT = np.ascontiguousarray(
            wo[:, 256 * g:256 * (g + 1)].T.astype(ml_dtypes.bfloat16))
        xT = np.ascontiguousarray(np.asarray(x)[b].T.astype(ml_dtypes.bfloat16))
        in_maps.append({"xT": xT, "wqkvT": wqkvT, "woT": woT,
                        "cc": cc, "ss": ss})
    return in_maps


def kernel(x, cos, sin, wq, wk, wv, wo, qn_w, kn_w):
    from concourse.bass_utils import run_bass_kernel_spmd

    if "nc" not in _CACHE:
        _CACHE["nc"] = _build_nc()
    nc = _CACHE["nc"]
    in_maps = make_in_maps(np.asarray(x), np.asarray(cos), np.asarray(sin),
                           np.asarray(wq), np.asarray(wk), np.asarray(wv),
                           np.asarray(wo), np.asarray(qn_w), np.asarray(kn_w))
    res = run_bass_kernel_spmd(nc, in_maps, core_ids=list(range(8)))
    out = np.zeros((2, T, D), np.float32)
    for core in range(8):
        b = core // 4
        out[b] += res.results[core]["out"].astype(np.float32)
    return out
